# revision 21
# baseline (speedup 1.0000x reference)
"""Trainium2 Bass kernel for a binarized-weight BasicBlock (dense CNN).

Reference computation (all fp32):
    out = clip(bn2(conv3x3(quant(clip(bn1(conv3x3(quant(x), sign(w1))), -1, 1)),
                  sign(w2)) + x), -1, 1)
with quant(v) = round-half-up(v * 128) / 128 and bn in inference form.

Strategy:
  * Data-parallel: batch 32 is sharded 4 images per NeuronCore across 8 cores.
  * Channels (256) live on partitions as 2 blocks of 128.
  * conv1 = 18 accumulating fp16 matmuls per output tile (9 taps x 2 input
    channel blocks), fp16 operands / fp32 PSUM accumulation.  Activations are
    integers k = 128*quant(v) with |k| <= ~730 and weights are +-1, so every
    product and partial sum is exactly representable: the fp16 matmul path is
    bit-exact, and PSUM holds 128*conv exactly.
  * conv2 = 9 accumulating fp8e4 DoubleRow matmuls per output tile: the two
    128-channel input blocks are packed into the DoubleRow pair axis, so one
    matmul contracts all 256 channels of a tap at 1 output column per cycle
    (2x the fp16 rate).  conv2's input k2 = 128*quant(clip(bn1,...)) is an
    integer in [-128, 128]; 98.3% of values saturate to +-128 (exact in
    e4m3) and the rest round with |err| <= 4, giving ~1.1e-2 final relative
    error (vs the 2e-2 tolerance).  The moving AP is [128, 2, 8, 56]: pair
    axis stride PLANEP (%16, a DoubleRow requirement), then the usual
    strided row window.
  * Activations are staged in zero-padded SBUF tiles; a conv matmul's moving
    operand is a window into the padded plane, so no shift DMAs are needed.
  * quantize is exact: z = 128*v + 0.5 (exact in fp32), t = RNE(z) via the
    +-1.5*2^23 magic add, floor(z) = t - (t > z).  Matches the reference's
    round-half-up tie behaviour bit-for-bit.
  * BN is folded host-side to per-channel (inv, bias) fp32 pairs; the device
    applies psum*(inv/128) + bias with the same fp32 rounding sequence as the
    reference.
"""

import numpy as np

_N = 32          # full batch
_C = 256         # channels
_H = 56          # height
_W = 56          # width
_NCORES = 8
_EPS = 1e-5

_cache = {}


def _build(n_img, C, H, W, RG):
    """Build + compile the per-core Bass program (SPMD, one NEFF for all cores)."""
    from contextlib import ExitStack

    import concourse.tile as tile
    from concourse import bacc, mybir

    F32 = mybir.dt.float32
    F16 = mybir.dt.float16
    F8 = mybir.dt.float8e4
    Alu = mybir.AluOpType
    Act = mybir.ActivationFunctionType
    DR = mybir.MatmulPerfMode.DoubleRow

    MAGIC = float(3 << 22)  # 1.5 * 2**23: RNE-to-integer for |z| < 2**22

    nblk = C // 128
    ngrp = H // RG
    HP, WP = H + 2, W + 2
    NW = 9 * nblk * nblk          # weight tiles per conv (36)
    PLANE = HP * WP               # 3364
    PLANEP = (PLANE + 15) // 16 * 16   # 3376: padded plane, %16 pair stride
    P2SZ = nblk * PLANEP          # pad2 flat bytes per partition

    nc = bacc.Bacc("TRN2", target_bir_lowering=False, debug=False,
                   num_devices=_NCORES)

    x_d = nc.dram_tensor("x", [n_img, C, H * W], F32, kind="ExternalInput")
    w1_d = nc.dram_tensor("wq1", [128, NW, 128], F16, kind="ExternalInput")
    w2_d = nc.dram_tensor("wq2", [128, NW, 128], F8, kind="ExternalInput")
    c_d = nc.dram_tensor("coef", [128, nblk, 4], F32, kind="ExternalInput")
    o_d = nc.dram_tensor("out", [n_img, C, H * W], F32, kind="ExternalOutput")

    def quant_chain(pool, zsrc, zscale, dst_ap, shape):
        """dst = floor(zscale*zsrc + 0.5) as fp16; exact round-half-up."""
        z = pool.tile(shape, F32, tag="qz")
        nc.scalar.activation(z[:], zsrc, Act.Copy, bias=0.5, scale=zscale)
        t = pool.tile(shape, F32, tag="qt")
        nc.vector.tensor_scalar(t[:], z[:], MAGIC, -MAGIC, Alu.add, Alu.add)
        c = pool.tile(shape, F32, tag="qc")
        nc.vector.tensor_tensor(c[:], t[:], z[:], Alu.is_gt)
        nc.vector.tensor_tensor(dst_ap, t[:], c[:], Alu.subtract)

    with tile.TileContext(nc) as tc, ExitStack() as ctx:
        const = ctx.enter_context(tc.tile_pool(name="const", bufs=1))
        xin = ctx.enter_context(tc.tile_pool(name="xin", bufs=2))
        pads = ctx.enter_context(tc.tile_pool(name="pads", bufs=2))
        q1s = ctx.enter_context(tc.tile_pool(name="q1s", bufs=2))
        e1s = ctx.enter_context(tc.tile_pool(name="e1s", bufs=3))
        e2s = ctx.enter_context(tc.tile_pool(name="e2s", bufs=6))
        psum = ctx.enter_context(tc.tile_pool(name="psum", bufs=7, space="PSUM"))
        warmp = ctx.enter_context(tc.tile_pool(name="warmp", bufs=1,
                                               space="PSUM"))

        # weight tiles: conv1 fp16 [ob][tap][ib], conv2 fp8 DoubleRow pairs
        # [ob][tap][pair=ib].  The first 4 conv1 tiles go in a mini-DMA so the
        # warm-up matmuls can start early.
        wt1 = const.tile([128, NW, 128], F16)
        nc.sync.dma_start(wt1[:, 0:4, :], w1_d.ap()[:, 0:4, :])

        wt2 = const.tile([128, NW, 128], F8)

        ct = const.tile([128, nblk, 4], F32)

        # image 0 in three DMA chunks (rows 0-9 gate the first matmul
        # group); quant chains stay piece-granular for early starts
        xi0 = x_d.ap()[0].rearrange("(b p) f -> p b f", p=128)
        x0_pieces = [[(0, 4), (4, 4)],
                     [(0, 2), (2, 6)],
                     [(0, 2), (2, 6)]] + \
            [[(0, 4), (4, 4)]] * (ngrp - 3)
        x0_pieces = x0_pieces[:ngrp]
        xt0 = xin.tile([128, nblk, H * W], F32, tag="x", name="x0")
        # chunks aligned to the quant pieces so no chain waits on a later
        # bulk transfer
        for (a, b) in ((0, 4), (4, 8)):
            nc.sync.dma_start(xt0[:, :, a * W:b * W], xi0[:, :, a * W:b * W])
        nc.sync.dma_start(ct[:], c_d.ap())
        nc.sync.dma_start(xt0[:, :, 8 * W:10 * W], xi0[:, :, 8 * W:10 * W])
        nc.sync.dma_start(wt1[:, 4:18, :], w1_d.ap()[:, 4:18, :])
        nc.sync.dma_start(xt0[:, :, 10 * W:32 * W], xi0[:, :, 10 * W:32 * W])
        nc.sync.dma_start(wt1[:, 18:NW, :], w1_d.ap()[:, 18:NW, :])
        nc.sync.dma_start(xt0[:, :, 32 * W:H * W], xi0[:, :, 32 * W:H * W])
        nc.sync.dma_start(wt2[:, 0:18, :], w2_d.ap()[:, 0:18, :])
        nc.sync.dma_start(wt2[:, 18:NW, :], w2_d.ap()[:, 18:NW, :])

        # dummy matmuls on the first mini-chunk: keeps the PE activity
        # monitor busy during the input fill so the real stream starts at
        # the full 2.4GHz clock
        warm = warmp.tile([128, 128], F32)
        for j in range(36):
            nc.tensor.matmul(warm[:], wt1[:, 0, :], wt1[:, j % 4, :],
                             start=True, stop=True)

        def conv1_mms(ps, pad, ob, r0, rg):
            """18 accumulating fp16 matmuls for output block ob, rows r0:+rg."""
            for tap in range(9):
                dy, dx = tap // 3 - 1, tap % 3 - 1
                for ib in range(nblk):
                    widx = (ob * 9 + tap) * nblk + ib
                    rhs = pad[:, ib, 1 + r0 + dy:1 + r0 + dy + rg,
                              1 + dx:1 + dx + W]
                    nc.tensor.matmul(ps[:, :rg * W], wt1[:, widx, :], rhs,
                                     start=(tap == 0 and ib == 0),
                                     stop=(tap == 8 and ib == nblk - 1))

        for i in range(n_img):
            if i == 0:
                xt = xt0
            else:
                xi = x_d.ap()[i].rearrange("(b p) f -> p b f", p=128)
                xt = xin.tile([128, nblk, H * W], F32, tag="x")
                nc.sync.dma_start(xt[:], xi)

            def xg(g):
                return xt[:, :, g * RG * W:(g + 1) * RG * W]

            # quantize input into padded conv1 operand (fp16, 4D tile)
            pad1 = pads.tile([128, nblk, HP, WP], F16, tag="pad1")
            nc.vector.memset(pad1[:, :, 0, :], 0.0)
            nc.vector.memset(pad1[:, :, HP - 1, :], 0.0)
            nc.vector.memset(pad1[:, :, 1:HP - 1, 0:1], 0.0)
            nc.vector.memset(pad1[:, :, 1:HP - 1, WP - 1:WP], 0.0)
            for g in range(ngrp):
                if i == 0:
                    # pieces matching the split DMAs
                    for (pr, pn) in x0_pieces[g]:
                        rows = slice(pr * W, (pr + pn) * W)
                        dst = pad1[:, :,
                                   1 + g * RG + pr:1 + g * RG + pr + pn,
                                   1:1 + W]
                        quant_chain(
                            q1s,
                            xg(g)[:, :, rows].rearrange(
                                "p b (h w) -> p b h w", w=W),
                            128.0, dst, [128, nblk, pn, W])
                else:
                    dst = pad1[:, :, 1 + g * RG:1 + (g + 1) * RG, 1:1 + W]
                    quant_chain(
                        q1s,
                        xg(g).rearrange("p b (h w) -> p b h w", w=W),
                        128.0, dst, [128, nblk, RG, W])

            # conv2 operand: fp8 planes at %16 pair stride (DoubleRow req.)
            pad2 = pads.tile([128, P2SZ], F8, tag="pad2")
            p2v4 = pad2[:].rearrange("p (b f) -> p b f", b=nblk)[
                :, :, 0:PLANE].rearrange("p b (h w) -> p b h w", w=WP)
            nc.vector.memset(p2v4[:, :, 0, :], 0.0)
            nc.vector.memset(p2v4[:, :, HP - 1, :], 0.0)
            nc.vector.memset(p2v4[:, :, 1:HP - 1, 0:1], 0.0)
            nc.vector.memset(p2v4[:, :, 1:HP - 1, WP - 1:WP], 0.0)

            # conv1 -> bn1 -> hardtanh -> quantize into fp8 conv2 operand
            grps1 = [(g * RG, RG) for g in range(ngrp)]
            for ob in range(nblk):
                if i == 0 and ob == 0:
                    # a 3-row first group only needs the first quant piece
                    # (rows 0-3), so real matmuls start ~3us earlier
                    grps = [(0, 3), (3, 5)] + grps1[1:]
                else:
                    grps = grps1
                for (r0, rg) in grps:
                    ps = psum.tile([128, RG * W], F32, tag="ps")
                    conv1_mms(ps, pad1, ob, r0, rg)
                    psf = ps[:, :rg * W]
                    # z = 128*bn1 + 0.5 in one ACT op (psum = 128*conv, so
                    # scale=inv1, bias=128*bias1+0.5); matches XLA's fused
                    # multiply-add rounding bit-for-bit
                    n = rg * W
                    z = e1s.tile([128, RG * W], F32, tag="zb")
                    nc.scalar.activation(z[:, :n], psf, Act.Identity,
                                         bias=ct[:, ob, 1:2],
                                         scale=ct[:, ob, 0:1])
                    cl = e1s.tile([128, RG * W], F32, tag="cl")
                    nc.vector.tensor_scalar(cl[:, :n], z[:, :n], 128.5,
                                            -127.5, Alu.min, Alu.max)
                    t = e1s.tile([128, RG * W], F32, tag="qt")
                    nc.vector.tensor_scalar(t[:, :n], cl[:, :n], MAGIC,
                                            -MAGIC, Alu.add, Alu.add)
                    c = e1s.tile([128, RG * W], F32, tag="qc")
                    nc.vector.tensor_tensor(c[:, :n], t[:, :n], cl[:, :n],
                                            Alu.is_gt)
                    # fp8 cast on write (RNE); |k2| <= 128 < 240 never clips
                    dst = p2v4[:, ob, 1 + r0:1 + r0 + rg, 1:1 + W]
                    nc.vector.tensor_tensor(
                        dst, t[:, :n].rearrange("p (h w) -> p h w", w=W),
                        c[:, :n].rearrange("p (h w) -> p h w", w=W),
                        Alu.subtract)

            # conv2 (fp8 DoubleRow) -> +residual -> bn2 -> hardtanh -> out
            # moving AP: [128, 2(pair), rg, W] strided row windows
            for ob in range(nblk):
                for g in range(ngrp):
                    r0 = g * RG
                    ps = psum.tile([128, RG * W], F32, tag="ps")
                    for tap in range(9):
                        dy, dx = tap // 3 - 1, tap % 3 - 1
                        rhs = p2v4[:, :, 1 + r0 + dy:1 + r0 + dy + RG,
                                   1 + dx:1 + dx + W]
                        nc.tensor.matmul(
                            ps[:], wt2[:, (ob * 9 + tap) * nblk:
                                       (ob * 9 + tap) * nblk + nblk, :],
                            rhs, start=(tap == 0), stop=(tap == 8),
                            perf_mode=DR)
                    psf = ps[:]
                    res = xg(g)[:, ob, :]
                    bn = e2s.tile([128, RG * W], F32, tag="bn2")
                    oc = e2s.tile([128, RG * W], F32, tag="oc")
                    last = (i == n_img - 1 and ob == nblk - 1
                            and g == ngrp - 1)
                    if not last:
                        s_t = e2s.tile([128, RG * W], F32, tag="s")
                        nc.vector.scalar_tensor_tensor(
                            s_t[:], psf, 1.0 / 128.0, res,
                            Alu.mult, Alu.add)
                        nc.scalar.activation(bn[:], s_t[:], Act.Identity,
                                             bias=ct[:, ob, 3:4],
                                             scale=ct[:, ob, 2:3])
                        nc.vector.tensor_scalar(oc[:], bn[:],
                                                1.0, -1.0, Alu.min, Alu.max)
                        nc.sync.dma_start(
                            o_d.ap()[i, ob * 128:(ob + 1) * 128,
                                     g * RG * W:(g + 1) * RG * W],
                            oc[:])
                    else:
                        # exposed kernel tail: keep ACT off the critical
                        # path by pre-folding bn2 into the residual
                        # (resb = x*inv2 + bias2, computed during the MMs),
                        # then psum*(inv2/128) + resb -> clip -> DMA in
                        # halves.  Ulp-level deviation from the reference
                        # rounding sequence, far below tolerance.
                        resb = e2s.tile([128, RG * W], F32, tag="resb")
                        nc.scalar.activation(resb[:], res, Act.Identity,
                                             bias=ct[:, ob, 3:4],
                                             scale=ct[:, ob, 2:3])
                        iv = e2s.tile([128, 1], F32, tag="iv")
                        nc.vector.tensor_scalar(iv[:], ct[:, ob, 2:3],
                                                1.0 / 128.0, None, Alu.mult)
                        nh = RG * W // 2
                        for hsl in (slice(0, nh), slice(nh, RG * W)):
                            nc.vector.scalar_tensor_tensor(
                                bn[:, hsl], psf[:, hsl], iv[:],
                                resb[:, hsl], Alu.mult, Alu.add)
                            nc.vector.tensor_scalar(
                                oc[:, hsl], bn[:, hsl],
                                1.0, -1.0, Alu.min, Alu.max)
                            nc.sync.dma_start(
                                o_d.ap()[i, ob * 128:(ob + 1) * 128,
                                         g * RG * W:(g + 1) * RG * W][:, hsl],
                                oc[:, hsl])

    nc.compile()
    return nc


def _get_program(n_img, C, H, W, RG):
    key = (n_img, C, H, W, RG)
    if key not in _cache:
        _cache[key] = _build(n_img, C, H, W, RG)
    return _cache[key]


def _fold_bn(g, b, m, v):
    """Per-channel (inv, bias) in fp32, matching the reference's op sequence."""
    try:
        import jax

        with jax.default_device(jax.devices("cpu")[0]):
            inv = np.asarray(jax.jit(
                lambda g_, v_: g_ * jax.lax.rsqrt(v_ + _EPS), backend="cpu"
            )(g, v))
            bias = np.asarray(jax.jit(
                lambda b_, m_, i_: b_ - m_ * i_, backend="cpu"
            )(b, m, inv))
        return inv.astype(np.float32), bias.astype(np.float32)
    except Exception:
        inv = (g.astype(np.float32)
               * (np.float32(1.0) / np.sqrt(v.astype(np.float32)
                                            + np.float32(_EPS))))
        bias = b.astype(np.float32) - m.astype(np.float32) * inv
        return inv.astype(np.float32), bias.astype(np.float32)


def _prep_weights(w1, w2, C):
    """lhsT tiles [128, 36, 128], slot (ob*9+tap)*nblk+ib, i on partitions."""
    import ml_dtypes

    nblk = C // 128
    t1 = np.empty((128, 9 * nblk * nblk, 128), np.float16)
    t2 = np.empty((128, 9 * nblk * nblk, 128), ml_dtypes.float8_e4m3)
    for w, tiles, dt in ((w1, t1, np.float16),
                        (w2, t2, ml_dtypes.float8_e4m3)):
        wq = np.where(w >= 0, 1.0, -1.0).astype(np.float32)
        for ob in range(nblk):
            for tap in range(9):
                dy, dx = tap // 3, tap % 3
                for ib in range(nblk):
                    idx = (ob * 9 + tap) * nblk + ib
                    blk = wq[ob * 128:(ob + 1) * 128,
                             ib * 128:(ib + 1) * 128, dy, dx]
                    tiles[:, idx, :] = blk.T.astype(dt)
    return t1, t2


def _make_in_maps(x, w1, w2, g1, b1, m1, v1, g2, b2, m2, v2):
    n, C, H, W = x.shape
    n_img = n // _NCORES
    nblk = C // 128

    wq1, wq2 = _prep_weights(np.asarray(w1), np.asarray(w2), C)
    inv1, bias1 = _fold_bn(np.asarray(g1), np.asarray(b1),
                           np.asarray(m1), np.asarray(v1))
    inv2, bias2 = _fold_bn(np.asarray(g2), np.asarray(b2),
                           np.asarray(m2), np.asarray(v2))
    bias1z = np.float32(128.0) * bias1 + np.float32(0.5)
    coef = np.empty((128, nblk, 4), np.float32)
    for blk in range(nblk):
        sl = slice(blk * 128, (blk + 1) * 128)
        coef[:, blk, 0] = inv1[sl]
        coef[:, blk, 1] = bias1z[sl]
        coef[:, blk, 2] = inv2[sl]
        coef[:, blk, 3] = bias2[sl]

    xr = np.ascontiguousarray(np.asarray(x).reshape(n, C, H * W),
                              dtype=np.float32)
    return [
        {"x": xr[i * n_img:(i + 1) * n_img], "wq1": wq1, "wq2": wq2,
         "coef": coef}
        for i in range(_NCORES)
    ]


def _run(trace=False, **inputs):
    from concourse.bass_utils import run_bass_kernel_spmd

    n, C, H, W = inputs["x"].shape
    nc = _get_program(n // _NCORES, C, H, W, 8)
    in_maps = _make_in_maps(**inputs)
    res = run_bass_kernel_spmd(nc, in_maps, core_ids=list(range(_NCORES)),
                               trace=trace)
    out = np.concatenate([r["out"] for r in res.results], axis=0)
    return out.reshape(n, C, H, W), res


def kernel(x, w1, w2, g1, b1, m1, v1, g2, b2, m2, v2):
    try:
        out, _ = _run(x=x, w1=w1, w2=w2, g1=g1, b1=b1, m1=m1, v1=v1,
                      g2=g2, b2=b2, m2=m2, v2=v2)
    except Exception:
        # one retry: a fresh NEFF's first execution occasionally hits a
        # transient device error; the identical program then runs fine
        import time

        time.sleep(2.0)
        out, _ = _run(x=x, w1=w1, w2=w2, g1=g1, b1=b1, m1=m1, v1=v1,
                      g2=g2, b2=b2, m2=m2, v2=v2)
    return out


# revision 23
# speedup vs baseline: 1.0069x; 1.0069x over previous
"""Trainium2 Bass kernel for a binarized-weight BasicBlock (dense CNN).

Reference computation (all fp32):
    out = clip(bn2(conv3x3(quant(clip(bn1(conv3x3(quant(x), sign(w1))), -1, 1)),
                  sign(w2)) + x), -1, 1)
with quant(v) = round-half-up(v * 128) / 128 and bn in inference form.

Strategy:
  * Data-parallel: batch 32 is sharded 4 images per NeuronCore across 8 cores.
  * Channels (256) live on partitions as 2 blocks of 128.
  * conv1 = 18 accumulating fp16 matmuls per output tile (9 taps x 2 input
    channel blocks), fp16 operands / fp32 PSUM accumulation.  Activations are
    integers k = 128*quant(v) with |k| <= ~730 and weights are +-1, so every
    product and partial sum is exactly representable: the fp16 matmul path is
    bit-exact, and PSUM holds 128*conv exactly.
  * conv2 = 9 accumulating fp8e4 DoubleRow matmuls per output tile: the two
    128-channel input blocks are packed into the DoubleRow pair axis, so one
    matmul contracts all 256 channels of a tap at 1 output column per cycle
    (2x the fp16 rate).  conv2's input k2 = 128*quant(clip(bn1,...)) is an
    integer in [-128, 128]; 98.3% of values saturate to +-128 (exact in
    e4m3) and the rest round with |err| <= 4, giving ~1.1e-2 final relative
    error (vs the 2e-2 tolerance).  The moving AP is [128, 2, 8, 56]: pair
    axis stride PLANEP (%16, a DoubleRow requirement), then the usual
    strided row window.
  * Activations are staged in zero-padded SBUF tiles; a conv matmul's moving
    operand is a window into the padded plane, so no shift DMAs are needed.
  * quantize is exact: z = 128*v + 0.5 (exact in fp32), t = RNE(z) via the
    +-1.5*2^23 magic add, floor(z) = t - (t > z).  Matches the reference's
    round-half-up tie behaviour bit-for-bit.
  * BN is folded host-side to per-channel (inv, bias) fp32 pairs; the device
    applies psum*(inv/128) + bias with the same fp32 rounding sequence as the
    reference.
"""

import numpy as np

_N = 32          # full batch
_C = 256         # channels
_H = 56          # height
_W = 56          # width
_NCORES = 8
_EPS = 1e-5

_cache = {}


def _build(n_img, C, H, W, RG):
    """Build + compile the per-core Bass program (SPMD, one NEFF for all cores)."""
    from contextlib import ExitStack

    import concourse.tile as tile
    from concourse import bacc, mybir

    F32 = mybir.dt.float32
    F16 = mybir.dt.float16
    F8 = mybir.dt.float8e4
    Alu = mybir.AluOpType
    Act = mybir.ActivationFunctionType
    DR = mybir.MatmulPerfMode.DoubleRow

    MAGIC = float(3 << 22)  # 1.5 * 2**23: RNE-to-integer for |z| < 2**22

    nblk = C // 128
    ngrp = H // RG
    HP, WP = H + 2, W + 2
    NW = 9 * nblk * nblk          # weight tiles per conv (36)
    PLANE = HP * WP               # 3364
    PLANEP = (PLANE + 15) // 16 * 16   # 3376: padded plane, %16 pair stride
    P2SZ = nblk * PLANEP          # pad2 flat bytes per partition

    nc = bacc.Bacc("TRN2", target_bir_lowering=False, debug=False,
                   num_devices=_NCORES)

    x_d = nc.dram_tensor("x", [n_img, C, H * W], F32, kind="ExternalInput")
    w1_d = nc.dram_tensor("wq1", [128, NW, 128], F16, kind="ExternalInput")
    w2_d = nc.dram_tensor("wq2", [128, NW, 128], F8, kind="ExternalInput")
    c_d = nc.dram_tensor("coef", [128, nblk, 4], F32, kind="ExternalInput")
    o_d = nc.dram_tensor("out", [n_img, C, H * W], F32, kind="ExternalOutput")

    def quant_chain(pool, zsrc, zscale, dst_ap, shape):
        """dst = floor(zscale*zsrc + 0.5) as fp16; exact round-half-up."""
        z = pool.tile(shape, F32, tag="qz")
        nc.scalar.activation(z[:], zsrc, Act.Copy, bias=0.5, scale=zscale)
        t = pool.tile(shape, F32, tag="qt")
        nc.vector.tensor_scalar(t[:], z[:], MAGIC, -MAGIC, Alu.add, Alu.add)
        c = pool.tile(shape, F32, tag="qc")
        nc.vector.tensor_tensor(c[:], t[:], z[:], Alu.is_gt)
        nc.vector.tensor_tensor(dst_ap, t[:], c[:], Alu.subtract)

    with tile.TileContext(nc) as tc, ExitStack() as ctx:
        const = ctx.enter_context(tc.tile_pool(name="const", bufs=1))
        xin = ctx.enter_context(tc.tile_pool(name="xin", bufs=2))
        pads = ctx.enter_context(tc.tile_pool(name="pads", bufs=2))
        q1s = ctx.enter_context(tc.tile_pool(name="q1s", bufs=2))
        e1s = ctx.enter_context(tc.tile_pool(name="e1s", bufs=3))
        e2s = ctx.enter_context(tc.tile_pool(name="e2s", bufs=6))
        psum = ctx.enter_context(tc.tile_pool(name="psum", bufs=7, space="PSUM"))
        warmp = ctx.enter_context(tc.tile_pool(name="warmp", bufs=1,
                                               space="PSUM"))

        # weight tiles: conv1 fp16 [ob][tap][ib], conv2 fp8 DoubleRow pairs
        # [ob][tap][pair=ib].  The first 4 conv1 tiles go in a mini-DMA so the
        # warm-up matmuls can start early.
        wt1 = const.tile([128, NW, 128], F16)
        nc.sync.dma_start(wt1[:, 0:4, :], w1_d.ap()[:, 0:4, :])

        wt2 = const.tile([128, NW, 128], F8)

        ct = const.tile([128, nblk, 4], F32)

        # image 0 in three DMA chunks (rows 0-9 gate the first matmul
        # group); quant chains stay piece-granular for early starts
        xi0 = x_d.ap()[0].rearrange("(b p) f -> p b f", p=128)
        x0_pieces = [[(0, 4), (4, 4)],
                     [(0, 2), (2, 6)],
                     [(0, 2), (2, 6)]] + \
            [[(0, 4), (4, 4)]] * (ngrp - 3)
        x0_pieces = x0_pieces[:ngrp]
        xt0 = xin.tile([128, nblk, H * W], F32, tag="x", name="x0")
        # chunks aligned to the quant pieces so no chain waits on a later
        # bulk transfer
        for (a, b) in ((0, 4), (4, 8)):
            nc.sync.dma_start(xt0[:, :, a * W:b * W], xi0[:, :, a * W:b * W])
        nc.sync.dma_start(ct[:], c_d.ap())
        nc.sync.dma_start(xt0[:, :, 8 * W:10 * W], xi0[:, :, 8 * W:10 * W])
        nc.sync.dma_start(wt1[:, 4:18, :], w1_d.ap()[:, 4:18, :])
        nc.sync.dma_start(xt0[:, :, 10 * W:32 * W], xi0[:, :, 10 * W:32 * W])
        nc.sync.dma_start(wt1[:, 18:NW, :], w1_d.ap()[:, 18:NW, :])
        nc.sync.dma_start(xt0[:, :, 32 * W:H * W], xi0[:, :, 32 * W:H * W])
        nc.sync.dma_start(wt2[:, 0:18, :], w2_d.ap()[:, 0:18, :])
        nc.sync.dma_start(wt2[:, 18:NW, :], w2_d.ap()[:, 18:NW, :])

        # dummy matmuls on the first mini-chunk: keeps the PE activity
        # monitor busy during the input fill so the real stream starts at
        # the full 2.4GHz clock
        warm = warmp.tile([128, 128], F32)
        for j in range(52):
            nc.tensor.matmul(warm[:], wt1[:, 0, :], wt1[:, j % 4, :],
                             start=True, stop=True)

        def conv1_mms(ps, pad, ob, r0, rg):
            """18 accumulating fp16 matmuls for output block ob, rows r0:+rg."""
            for tap in range(9):
                dy, dx = tap // 3 - 1, tap % 3 - 1
                for ib in range(nblk):
                    widx = (ob * 9 + tap) * nblk + ib
                    rhs = pad[:, ib, 1 + r0 + dy:1 + r0 + dy + rg,
                              1 + dx:1 + dx + W]
                    nc.tensor.matmul(ps[:, :rg * W], wt1[:, widx, :], rhs,
                                     start=(tap == 0 and ib == 0),
                                     stop=(tap == 8 and ib == nblk - 1))

        for i in range(n_img):
            if i == 0:
                xt = xt0
            else:
                xi = x_d.ap()[i].rearrange("(b p) f -> p b f", p=128)
                xt = xin.tile([128, nblk, H * W], F32, tag="x")
                nc.sync.dma_start(xt[:], xi)

            def xg(g):
                return xt[:, :, g * RG * W:(g + 1) * RG * W]

            # quantize input into padded conv1 operand (fp16, 4D tile)
            pad1 = pads.tile([128, nblk, HP, WP], F16, tag="pad1")
            nc.vector.memset(pad1[:, :, 0, :], 0.0)
            nc.vector.memset(pad1[:, :, HP - 1, :], 0.0)
            nc.vector.memset(pad1[:, :, 1:HP - 1, 0:1], 0.0)
            nc.vector.memset(pad1[:, :, 1:HP - 1, WP - 1:WP], 0.0)
            for g in range(ngrp):
                if i == 0:
                    # pieces matching the split DMAs
                    for (pr, pn) in x0_pieces[g]:
                        rows = slice(pr * W, (pr + pn) * W)
                        dst = pad1[:, :,
                                   1 + g * RG + pr:1 + g * RG + pr + pn,
                                   1:1 + W]
                        quant_chain(
                            q1s,
                            xg(g)[:, :, rows].rearrange(
                                "p b (h w) -> p b h w", w=W),
                            128.0, dst, [128, nblk, pn, W])
                else:
                    dst = pad1[:, :, 1 + g * RG:1 + (g + 1) * RG, 1:1 + W]
                    quant_chain(
                        q1s,
                        xg(g).rearrange("p b (h w) -> p b h w", w=W),
                        128.0, dst, [128, nblk, RG, W])

            # conv2 operand: fp8 planes at %16 pair stride (DoubleRow req.)
            pad2 = pads.tile([128, P2SZ], F8, tag="pad2")
            p2v4 = pad2[:].rearrange("p (b f) -> p b f", b=nblk)[
                :, :, 0:PLANE].rearrange("p b (h w) -> p b h w", w=WP)
            nc.vector.memset(p2v4[:, :, 0, :], 0.0)
            nc.vector.memset(p2v4[:, :, HP - 1, :], 0.0)
            nc.vector.memset(p2v4[:, :, 1:HP - 1, 0:1], 0.0)
            nc.vector.memset(p2v4[:, :, 1:HP - 1, WP - 1:WP], 0.0)

            # conv1 -> bn1 -> hardtanh -> quantize into fp8 conv2 operand
            grps1 = [(g * RG, RG) for g in range(ngrp)]
            for ob in range(nblk):
                for (r0, rg) in grps1:
                    ps = psum.tile([128, RG * W], F32, tag="ps")
                    conv1_mms(ps, pad1, ob, r0, rg)
                    psf = ps[:, :rg * W]
                    # z = 128*bn1 + 0.5 in one ACT op (psum = 128*conv, so
                    # scale=inv1, bias=128*bias1+0.5); matches XLA's fused
                    # multiply-add rounding bit-for-bit
                    n = rg * W
                    z = e1s.tile([128, RG * W], F32, tag="zb")
                    nc.scalar.activation(z[:, :n], psf, Act.Identity,
                                         bias=ct[:, ob, 1:2],
                                         scale=ct[:, ob, 0:1])
                    cl = e1s.tile([128, RG * W], F32, tag="cl")
                    nc.vector.tensor_scalar(cl[:, :n], z[:, :n], 128.5,
                                            -127.5, Alu.min, Alu.max)
                    t = e1s.tile([128, RG * W], F32, tag="qt")
                    nc.vector.tensor_scalar(t[:, :n], cl[:, :n], MAGIC,
                                            -MAGIC, Alu.add, Alu.add)
                    c = e1s.tile([128, RG * W], F32, tag="qc")
                    nc.vector.tensor_tensor(c[:, :n], t[:, :n], cl[:, :n],
                                            Alu.is_gt)
                    # fp8 cast on write (RNE); |k2| <= 128 < 240 never clips
                    dst = p2v4[:, ob, 1 + r0:1 + r0 + rg, 1:1 + W]
                    nc.vector.tensor_tensor(
                        dst, t[:, :n].rearrange("p (h w) -> p h w", w=W),
                        c[:, :n].rearrange("p (h w) -> p h w", w=W),
                        Alu.subtract)

            # conv2 (fp8 DoubleRow) -> +residual -> bn2 -> hardtanh -> out
            # moving AP: [128, 2(pair), rg, W] strided row windows
            for ob in range(nblk):
                for g in range(ngrp):
                    r0 = g * RG
                    ps = psum.tile([128, RG * W], F32, tag="ps")
                    for tap in range(9):
                        dy, dx = tap // 3 - 1, tap % 3 - 1
                        rhs = p2v4[:, :, 1 + r0 + dy:1 + r0 + dy + RG,
                                   1 + dx:1 + dx + W]
                        nc.tensor.matmul(
                            ps[:], wt2[:, (ob * 9 + tap) * nblk:
                                       (ob * 9 + tap) * nblk + nblk, :],
                            rhs, start=(tap == 0), stop=(tap == 8),
                            perf_mode=DR)
                    psf = ps[:]
                    res = xg(g)[:, ob, :]
                    bn = e2s.tile([128, RG * W], F32, tag="bn2")
                    oc = e2s.tile([128, RG * W], F32, tag="oc")
                    last = (i == n_img - 1 and ob == nblk - 1
                            and g == ngrp - 1)
                    if not last:
                        s_t = e2s.tile([128, RG * W], F32, tag="s")
                        nc.vector.scalar_tensor_tensor(
                            s_t[:], psf, 1.0 / 128.0, res,
                            Alu.mult, Alu.add)
                        nc.scalar.activation(bn[:], s_t[:], Act.Identity,
                                             bias=ct[:, ob, 3:4],
                                             scale=ct[:, ob, 2:3])
                        nc.vector.tensor_scalar(oc[:], bn[:],
                                                1.0, -1.0, Alu.min, Alu.max)
                        nc.sync.dma_start(
                            o_d.ap()[i, ob * 128:(ob + 1) * 128,
                                     g * RG * W:(g + 1) * RG * W],
                            oc[:])
                    else:
                        # exposed kernel tail: keep ACT off the critical
                        # path by pre-folding bn2 into the residual
                        # (resb = x*inv2 + bias2, computed during the MMs),
                        # then psum*(inv2/128) + resb -> clip -> DMA in
                        # halves.  Ulp-level deviation from the reference
                        # rounding sequence, far below tolerance.
                        resb = e2s.tile([128, RG * W], F32, tag="resb")
                        nc.scalar.activation(resb[:], res, Act.Identity,
                                             bias=ct[:, ob, 3:4],
                                             scale=ct[:, ob, 2:3])
                        iv = e2s.tile([128, 1], F32, tag="iv")
                        nc.vector.tensor_scalar(iv[:], ct[:, ob, 2:3],
                                                1.0 / 128.0, None, Alu.mult)
                        nh = RG * W // 2
                        for hsl in (slice(0, nh), slice(nh, RG * W)):
                            nc.vector.scalar_tensor_tensor(
                                bn[:, hsl], psf[:, hsl], iv[:],
                                resb[:, hsl], Alu.mult, Alu.add)
                            nc.vector.tensor_scalar(
                                oc[:, hsl], bn[:, hsl],
                                1.0, -1.0, Alu.min, Alu.max)
                            nc.sync.dma_start(
                                o_d.ap()[i, ob * 128:(ob + 1) * 128,
                                         g * RG * W:(g + 1) * RG * W][:, hsl],
                                oc[:, hsl])

    nc.compile()
    return nc


def _get_program(n_img, C, H, W, RG):
    key = (n_img, C, H, W, RG)
    if key not in _cache:
        _cache[key] = _build(n_img, C, H, W, RG)
    return _cache[key]


def _fold_bn(g, b, m, v):
    """Per-channel (inv, bias) in fp32, matching the reference's op sequence."""
    try:
        import jax

        with jax.default_device(jax.devices("cpu")[0]):
            inv = np.asarray(jax.jit(
                lambda g_, v_: g_ * jax.lax.rsqrt(v_ + _EPS), backend="cpu"
            )(g, v))
            bias = np.asarray(jax.jit(
                lambda b_, m_, i_: b_ - m_ * i_, backend="cpu"
            )(b, m, inv))
        return inv.astype(np.float32), bias.astype(np.float32)
    except Exception:
        inv = (g.astype(np.float32)
               * (np.float32(1.0) / np.sqrt(v.astype(np.float32)
                                            + np.float32(_EPS))))
        bias = b.astype(np.float32) - m.astype(np.float32) * inv
        return inv.astype(np.float32), bias.astype(np.float32)


def _prep_weights(w1, w2, C):
    """lhsT tiles [128, 36, 128], slot (ob*9+tap)*nblk+ib, i on partitions."""
    import ml_dtypes

    nblk = C // 128
    t1 = np.empty((128, 9 * nblk * nblk, 128), np.float16)
    t2 = np.empty((128, 9 * nblk * nblk, 128), ml_dtypes.float8_e4m3)
    for w, tiles, dt in ((w1, t1, np.float16),
                        (w2, t2, ml_dtypes.float8_e4m3)):
        wq = np.where(w >= 0, 1.0, -1.0).astype(np.float32)
        for ob in range(nblk):
            for tap in range(9):
                dy, dx = tap // 3, tap % 3
                for ib in range(nblk):
                    idx = (ob * 9 + tap) * nblk + ib
                    blk = wq[ob * 128:(ob + 1) * 128,
                             ib * 128:(ib + 1) * 128, dy, dx]
                    tiles[:, idx, :] = blk.T.astype(dt)
    return t1, t2


def _make_in_maps(x, w1, w2, g1, b1, m1, v1, g2, b2, m2, v2):
    n, C, H, W = x.shape
    n_img = n // _NCORES
    nblk = C // 128

    wq1, wq2 = _prep_weights(np.asarray(w1), np.asarray(w2), C)
    inv1, bias1 = _fold_bn(np.asarray(g1), np.asarray(b1),
                           np.asarray(m1), np.asarray(v1))
    inv2, bias2 = _fold_bn(np.asarray(g2), np.asarray(b2),
                           np.asarray(m2), np.asarray(v2))
    bias1z = np.float32(128.0) * bias1 + np.float32(0.5)
    coef = np.empty((128, nblk, 4), np.float32)
    for blk in range(nblk):
        sl = slice(blk * 128, (blk + 1) * 128)
        coef[:, blk, 0] = inv1[sl]
        coef[:, blk, 1] = bias1z[sl]
        coef[:, blk, 2] = inv2[sl]
        coef[:, blk, 3] = bias2[sl]

    xr = np.ascontiguousarray(np.asarray(x).reshape(n, C, H * W),
                              dtype=np.float32)
    return [
        {"x": xr[i * n_img:(i + 1) * n_img], "wq1": wq1, "wq2": wq2,
         "coef": coef}
        for i in range(_NCORES)
    ]


def _run(trace=False, **inputs):
    from concourse.bass_utils import run_bass_kernel_spmd

    n, C, H, W = inputs["x"].shape
    nc = _get_program(n // _NCORES, C, H, W, 8)
    in_maps = _make_in_maps(**inputs)
    res = run_bass_kernel_spmd(nc, in_maps, core_ids=list(range(_NCORES)),
                               trace=trace)
    out = np.concatenate([r["out"] for r in res.results], axis=0)
    return out.reshape(n, C, H, W), res


def kernel(x, w1, w2, g1, b1, m1, v1, g2, b2, m2, v2):
    try:
        out, _ = _run(x=x, w1=w1, w2=w2, g1=g1, b1=b1, m1=m1, v1=v1,
                      g2=g2, b2=b2, m2=m2, v2=v2)
    except Exception:
        # one retry: a fresh NEFF's first execution occasionally hits a
        # transient device error; the identical program then runs fine
        import time

        time.sleep(2.0)
        out, _ = _run(x=x, w1=w1, w2=w2, g1=g1, b1=b1, m1=m1, v1=v1,
                      g2=g2, b2=b2, m2=m2, v2=v2)
    return out


# revision 26
# speedup vs baseline: 1.0092x; 1.0023x over previous
"""Trainium2 Bass kernel for a binarized-weight BasicBlock (dense CNN).

Reference computation (all fp32):
    out = clip(bn2(conv3x3(quant(clip(bn1(conv3x3(quant(x), sign(w1))), -1, 1)),
                  sign(w2)) + x), -1, 1)
with quant(v) = round-half-up(v * 128) / 128 and bn in inference form.

Strategy:
  * Data-parallel: batch 32 is sharded 4 images per NeuronCore across 8 cores.
  * Channels (256) live on partitions as 2 blocks of 128.
  * conv1 = 18 accumulating fp16 matmuls per output tile (9 taps x 2 input
    channel blocks), fp16 operands / fp32 PSUM accumulation.  Activations are
    integers k = 128*quant(v) with |k| <= ~730 and weights are +-1, so every
    product and partial sum is exactly representable: the fp16 matmul path is
    bit-exact, and PSUM holds 128*conv exactly.
  * conv2 = 9 accumulating fp8e4 DoubleRow matmuls per output tile: the two
    128-channel input blocks are packed into the DoubleRow pair axis, so one
    matmul contracts all 256 channels of a tap at 1 output column per cycle
    (2x the fp16 rate).  conv2's input k2 = 128*quant(clip(bn1,...)) is an
    integer in [-128, 128]; 98.3% of values saturate to +-128 (exact in
    e4m3) and the rest round with |err| <= 4, giving ~1.1e-2 final relative
    error (vs the 2e-2 tolerance).  The moving AP is [128, 2, 8, 56]: pair
    axis stride PLANEP (%16, a DoubleRow requirement), then the usual
    strided row window.
  * Activations are staged in zero-padded SBUF tiles; a conv matmul's moving
    operand is a window into the padded plane, so no shift DMAs are needed.
  * quantize is exact: z = 128*v + 0.5 (exact in fp32), t = RNE(z) via the
    +-1.5*2^23 magic add, floor(z) = t - (t > z).  Matches the reference's
    round-half-up tie behaviour bit-for-bit.
  * BN is folded host-side to per-channel (inv, bias) fp32 pairs; the device
    applies psum*(inv/128) + bias with the same fp32 rounding sequence as the
    reference.
"""

import numpy as np

_N = 32          # full batch
_C = 256         # channels
_H = 56          # height
_W = 56          # width
_NCORES = 8
_EPS = 1e-5

_cache = {}


def _build(n_img, C, H, W, RG):
    """Build + compile the per-core Bass program (SPMD, one NEFF for all cores)."""
    from contextlib import ExitStack

    import concourse.tile as tile
    from concourse import bacc, mybir

    F32 = mybir.dt.float32
    F16 = mybir.dt.float16
    F8 = mybir.dt.float8e4
    Alu = mybir.AluOpType
    Act = mybir.ActivationFunctionType
    DR = mybir.MatmulPerfMode.DoubleRow

    MAGIC = float(3 << 22)  # 1.5 * 2**23: RNE-to-integer for |z| < 2**22

    nblk = C // 128
    ngrp = H // RG
    HP, WP = H + 2, W + 2
    NW = 9 * nblk * nblk          # weight tiles per conv (36)
    PLANE = HP * WP               # 3364
    PLANEP = (PLANE + 15) // 16 * 16   # 3376: padded plane, %16 pair stride
    P2SZ = nblk * PLANEP          # pad2 flat bytes per partition

    nc = bacc.Bacc("TRN2", target_bir_lowering=False, debug=False,
                   num_devices=_NCORES)

    x_d = nc.dram_tensor("x", [n_img, C, H * W], F32, kind="ExternalInput")
    w1_d = nc.dram_tensor("wq1", [128, NW, 128], F16, kind="ExternalInput")
    w2_d = nc.dram_tensor("wq2", [128, NW, 128], F8, kind="ExternalInput")
    c_d = nc.dram_tensor("coef", [128, nblk, 4], F32, kind="ExternalInput")
    o_d = nc.dram_tensor("out", [n_img, C, H * W], F32, kind="ExternalOutput")

    def quant_chain(pool, zsrc, zscale, dst_ap, shape):
        """dst = floor(zscale*zsrc + 0.5) as fp16; exact round-half-up."""
        z = pool.tile(shape, F32, tag="qz")
        nc.scalar.activation(z[:], zsrc, Act.Copy, bias=0.5, scale=zscale)
        t = pool.tile(shape, F32, tag="qt")
        nc.vector.tensor_scalar(t[:], z[:], MAGIC, -MAGIC, Alu.add, Alu.add)
        c = pool.tile(shape, F32, tag="qc")
        nc.vector.tensor_tensor(c[:], t[:], z[:], Alu.is_gt)
        nc.vector.tensor_tensor(dst_ap, t[:], c[:], Alu.subtract)

    with tile.TileContext(nc) as tc, ExitStack() as ctx:
        const = ctx.enter_context(tc.tile_pool(name="const", bufs=1))
        xin = ctx.enter_context(tc.tile_pool(name="xin", bufs=2))
        pads = ctx.enter_context(tc.tile_pool(name="pads", bufs=2))
        q1s = ctx.enter_context(tc.tile_pool(name="q1s", bufs=2))
        e1s = ctx.enter_context(tc.tile_pool(name="e1s", bufs=3))
        e2s = ctx.enter_context(tc.tile_pool(name="e2s", bufs=6))
        psum = ctx.enter_context(tc.tile_pool(name="psum", bufs=7, space="PSUM"))
        warmp = ctx.enter_context(tc.tile_pool(name="warmp", bufs=1,
                                               space="PSUM"))

        # weight tiles: conv1 fp16 [ob][tap][ib], conv2 fp8 DoubleRow pairs
        # [ob][tap][pair=ib].  The first 4 conv1 tiles go in a mini-DMA so the
        # warm-up matmuls can start early.
        wt1 = const.tile([128, NW, 128], F16)
        nc.sync.dma_start(wt1[:, 0:4, :], w1_d.ap()[:, 0:4, :])

        wt2 = const.tile([128, NW, 128], F8)

        ct = const.tile([128, nblk, 4], F32)

        # image 0 in three DMA chunks (rows 0-9 gate the first matmul
        # group); quant chains stay piece-granular for early starts
        xi0 = x_d.ap()[0].rearrange("(b p) f -> p b f", p=128)
        x0_pieces = [[(0, 4), (4, 4)],
                     [(0, 2), (2, 6)],
                     [(0, 2), (2, 6)]] + \
            [[(0, 4), (4, 4)]] * (ngrp - 3)
        x0_pieces = x0_pieces[:ngrp]
        xt0 = xin.tile([128, nblk, H * W], F32, tag="x", name="x0")
        nc.sync.dma_start(xt0[:, :, 0:10 * W], xi0[:, :, 0:10 * W])
        nc.sync.dma_start(ct[:], c_d.ap())
        nc.sync.dma_start(xt0[:, :, 10 * W:32 * W], xi0[:, :, 10 * W:32 * W])
        nc.sync.dma_start(wt1[:, 4:18, :], w1_d.ap()[:, 4:18, :])
        nc.sync.dma_start(xt0[:, :, 32 * W:H * W], xi0[:, :, 32 * W:H * W])
        nc.sync.dma_start(wt1[:, 18:NW, :], w1_d.ap()[:, 18:NW, :])
        nc.sync.dma_start(wt2[:, 0:18, :], w2_d.ap()[:, 0:18, :])
        nc.sync.dma_start(wt2[:, 18:NW, :], w2_d.ap()[:, 18:NW, :])

        # dummy matmuls on the first mini-chunk: keeps the PE activity
        # monitor busy during the input fill so the real stream starts at
        # the full 2.4GHz clock
        warm = warmp.tile([128, 128], F32)
        for j in range(56):
            nc.tensor.matmul(warm[:], wt1[:, 0, :], wt1[:, j % 4, :],
                             start=True, stop=True)

        def conv1_mms(ps, pad, ob, r0, rg):
            """18 accumulating fp16 matmuls for output block ob, rows r0:+rg."""
            for tap in range(9):
                dy, dx = tap // 3 - 1, tap % 3 - 1
                for ib in range(nblk):
                    widx = (ob * 9 + tap) * nblk + ib
                    rhs = pad[:, ib, 1 + r0 + dy:1 + r0 + dy + rg,
                              1 + dx:1 + dx + W]
                    nc.tensor.matmul(ps[:, :rg * W], wt1[:, widx, :], rhs,
                                     start=(tap == 0 and ib == 0),
                                     stop=(tap == 8 and ib == nblk - 1))

        for i in range(n_img):
            if i == 0:
                xt = xt0
            else:
                xi = x_d.ap()[i].rearrange("(b p) f -> p b f", p=128)
                xt = xin.tile([128, nblk, H * W], F32, tag="x")
                nc.sync.dma_start(xt[:], xi)

            def xg(g):
                return xt[:, :, g * RG * W:(g + 1) * RG * W]

            # quantize input into padded conv1 operand (fp16, 4D tile)
            pad1 = pads.tile([128, nblk, HP, WP], F16, tag="pad1")
            nc.vector.memset(pad1[:, :, 0, :], 0.0)
            nc.vector.memset(pad1[:, :, HP - 1, :], 0.0)
            nc.vector.memset(pad1[:, :, 1:HP - 1, 0:1], 0.0)
            nc.vector.memset(pad1[:, :, 1:HP - 1, WP - 1:WP], 0.0)
            for g in range(ngrp):
                if i == 0:
                    # pieces matching the split DMAs
                    for (pr, pn) in x0_pieces[g]:
                        rows = slice(pr * W, (pr + pn) * W)
                        dst = pad1[:, :,
                                   1 + g * RG + pr:1 + g * RG + pr + pn,
                                   1:1 + W]
                        quant_chain(
                            q1s,
                            xg(g)[:, :, rows].rearrange(
                                "p b (h w) -> p b h w", w=W),
                            128.0, dst, [128, nblk, pn, W])
                else:
                    dst = pad1[:, :, 1 + g * RG:1 + (g + 1) * RG, 1:1 + W]
                    quant_chain(
                        q1s,
                        xg(g).rearrange("p b (h w) -> p b h w", w=W),
                        128.0, dst, [128, nblk, RG, W])

            # conv2 operand: fp8 planes at %16 pair stride (DoubleRow req.)
            pad2 = pads.tile([128, P2SZ], F8, tag="pad2")
            p2v4 = pad2[:].rearrange("p (b f) -> p b f", b=nblk)[
                :, :, 0:PLANE].rearrange("p b (h w) -> p b h w", w=WP)
            nc.vector.memset(p2v4[:, :, 0, :], 0.0)
            nc.vector.memset(p2v4[:, :, HP - 1, :], 0.0)
            nc.vector.memset(p2v4[:, :, 1:HP - 1, 0:1], 0.0)
            nc.vector.memset(p2v4[:, :, 1:HP - 1, WP - 1:WP], 0.0)

            # conv1 -> bn1 -> hardtanh -> quantize into fp8 conv2 operand
            grps1 = [(g * RG, RG) for g in range(ngrp)]
            for ob in range(nblk):
                for (r0, rg) in grps1:
                    ps = psum.tile([128, RG * W], F32, tag="ps")
                    conv1_mms(ps, pad1, ob, r0, rg)
                    psf = ps[:, :rg * W]
                    # z = 128*bn1 + 0.5 in one ACT op (psum = 128*conv, so
                    # scale=inv1, bias=128*bias1+0.5); matches XLA's fused
                    # multiply-add rounding bit-for-bit
                    n = rg * W
                    z = e1s.tile([128, RG * W], F32, tag="zb")
                    nc.scalar.activation(z[:, :n], psf, Act.Identity,
                                         bias=ct[:, ob, 1:2],
                                         scale=ct[:, ob, 0:1])
                    cl = e1s.tile([128, RG * W], F32, tag="cl")
                    nc.vector.tensor_scalar(cl[:, :n], z[:, :n], 128.5,
                                            -127.5, Alu.min, Alu.max)
                    t = e1s.tile([128, RG * W], F32, tag="qt")
                    nc.vector.tensor_scalar(t[:, :n], cl[:, :n], MAGIC,
                                            -MAGIC, Alu.add, Alu.add)
                    c = e1s.tile([128, RG * W], F32, tag="qc")
                    nc.vector.tensor_tensor(c[:, :n], t[:, :n], cl[:, :n],
                                            Alu.is_gt)
                    # fp8 cast on write (RNE); |k2| <= 128 < 240 never clips
                    dst = p2v4[:, ob, 1 + r0:1 + r0 + rg, 1:1 + W]
                    nc.vector.tensor_tensor(
                        dst, t[:, :n].rearrange("p (h w) -> p h w", w=W),
                        c[:, :n].rearrange("p (h w) -> p h w", w=W),
                        Alu.subtract)

            # conv2 (fp8 DoubleRow) -> +residual -> bn2 -> hardtanh -> out
            # moving AP: [128, 2(pair), rg, W] strided row windows
            for ob in range(nblk):
                for g in range(ngrp):
                    r0 = g * RG
                    ps = psum.tile([128, RG * W], F32, tag="ps")
                    for tap in range(9):
                        dy, dx = tap // 3 - 1, tap % 3 - 1
                        rhs = p2v4[:, :, 1 + r0 + dy:1 + r0 + dy + RG,
                                   1 + dx:1 + dx + W]
                        nc.tensor.matmul(
                            ps[:], wt2[:, (ob * 9 + tap) * nblk:
                                       (ob * 9 + tap) * nblk + nblk, :],
                            rhs, start=(tap == 0), stop=(tap == 8),
                            perf_mode=DR)
                    psf = ps[:]
                    res = xg(g)[:, ob, :]
                    bn = e2s.tile([128, RG * W], F32, tag="bn2")
                    oc = e2s.tile([128, RG * W], F32, tag="oc")
                    last = (i == n_img - 1 and ob == nblk - 1
                            and g == ngrp - 1)
                    if not last:
                        s_t = e2s.tile([128, RG * W], F32, tag="s")
                        nc.vector.scalar_tensor_tensor(
                            s_t[:], psf, 1.0 / 128.0, res,
                            Alu.mult, Alu.add)
                        nc.scalar.activation(bn[:], s_t[:], Act.Identity,
                                             bias=ct[:, ob, 3:4],
                                             scale=ct[:, ob, 2:3])
                        nc.vector.tensor_scalar(oc[:], bn[:],
                                                1.0, -1.0, Alu.min, Alu.max)
                        nc.sync.dma_start(
                            o_d.ap()[i, ob * 128:(ob + 1) * 128,
                                     g * RG * W:(g + 1) * RG * W],
                            oc[:])
                    else:
                        # exposed kernel tail: keep ACT off the critical
                        # path by pre-folding bn2 into the residual
                        # (resb = x*inv2 + bias2, computed during the MMs),
                        # then psum*(inv2/128) + resb -> clip -> DMA in
                        # halves.  Ulp-level deviation from the reference
                        # rounding sequence, far below tolerance.
                        resb = e2s.tile([128, RG * W], F32, tag="resb")
                        nc.scalar.activation(resb[:], res, Act.Identity,
                                             bias=ct[:, ob, 3:4],
                                             scale=ct[:, ob, 2:3])
                        iv = e2s.tile([128, 1], F32, tag="iv")
                        nc.vector.tensor_scalar(iv[:], ct[:, ob, 2:3],
                                                1.0 / 128.0, None, Alu.mult)
                        nh = RG * W // 2
                        for hsl in (slice(0, nh), slice(nh, RG * W)):
                            nc.vector.scalar_tensor_tensor(
                                bn[:, hsl], psf[:, hsl], iv[:],
                                resb[:, hsl], Alu.mult, Alu.add)
                            nc.vector.tensor_scalar(
                                oc[:, hsl], bn[:, hsl],
                                1.0, -1.0, Alu.min, Alu.max)
                            nc.sync.dma_start(
                                o_d.ap()[i, ob * 128:(ob + 1) * 128,
                                         g * RG * W:(g + 1) * RG * W][:, hsl],
                                oc[:, hsl])

    nc.compile()
    return nc


def _get_program(n_img, C, H, W, RG):
    key = (n_img, C, H, W, RG)
    if key not in _cache:
        _cache[key] = _build(n_img, C, H, W, RG)
    return _cache[key]


def _fold_bn(g, b, m, v):
    """Per-channel (inv, bias) in fp32, matching the reference's op sequence."""
    try:
        import jax

        with jax.default_device(jax.devices("cpu")[0]):
            inv = np.asarray(jax.jit(
                lambda g_, v_: g_ * jax.lax.rsqrt(v_ + _EPS), backend="cpu"
            )(g, v))
            bias = np.asarray(jax.jit(
                lambda b_, m_, i_: b_ - m_ * i_, backend="cpu"
            )(b, m, inv))
        return inv.astype(np.float32), bias.astype(np.float32)
    except Exception:
        inv = (g.astype(np.float32)
               * (np.float32(1.0) / np.sqrt(v.astype(np.float32)
                                            + np.float32(_EPS))))
        bias = b.astype(np.float32) - m.astype(np.float32) * inv
        return inv.astype(np.float32), bias.astype(np.float32)


def _prep_weights(w1, w2, C):
    """lhsT tiles [128, 36, 128], slot (ob*9+tap)*nblk+ib, i on partitions."""
    import ml_dtypes

    nblk = C // 128
    t1 = np.empty((128, 9 * nblk * nblk, 128), np.float16)
    t2 = np.empty((128, 9 * nblk * nblk, 128), ml_dtypes.float8_e4m3)
    for w, tiles, dt in ((w1, t1, np.float16),
                        (w2, t2, ml_dtypes.float8_e4m3)):
        wq = np.where(w >= 0, 1.0, -1.0).astype(np.float32)
        for ob in range(nblk):
            for tap in range(9):
                dy, dx = tap // 3, tap % 3
                for ib in range(nblk):
                    idx = (ob * 9 + tap) * nblk + ib
                    blk = wq[ob * 128:(ob + 1) * 128,
                             ib * 128:(ib + 1) * 128, dy, dx]
                    tiles[:, idx, :] = blk.T.astype(dt)
    return t1, t2


def _make_in_maps(x, w1, w2, g1, b1, m1, v1, g2, b2, m2, v2):
    n, C, H, W = x.shape
    n_img = n // _NCORES
    nblk = C // 128

    wq1, wq2 = _prep_weights(np.asarray(w1), np.asarray(w2), C)
    inv1, bias1 = _fold_bn(np.asarray(g1), np.asarray(b1),
                           np.asarray(m1), np.asarray(v1))
    inv2, bias2 = _fold_bn(np.asarray(g2), np.asarray(b2),
                           np.asarray(m2), np.asarray(v2))
    bias1z = np.float32(128.0) * bias1 + np.float32(0.5)
    coef = np.empty((128, nblk, 4), np.float32)
    for blk in range(nblk):
        sl = slice(blk * 128, (blk + 1) * 128)
        coef[:, blk, 0] = inv1[sl]
        coef[:, blk, 1] = bias1z[sl]
        coef[:, blk, 2] = inv2[sl]
        coef[:, blk, 3] = bias2[sl]

    xr = np.ascontiguousarray(np.asarray(x).reshape(n, C, H * W),
                              dtype=np.float32)
    return [
        {"x": xr[i * n_img:(i + 1) * n_img], "wq1": wq1, "wq2": wq2,
         "coef": coef}
        for i in range(_NCORES)
    ]


def _run(trace=False, **inputs):
    from concourse.bass_utils import run_bass_kernel_spmd

    n, C, H, W = inputs["x"].shape
    nc = _get_program(n // _NCORES, C, H, W, 8)
    in_maps = _make_in_maps(**inputs)
    res = run_bass_kernel_spmd(nc, in_maps, core_ids=list(range(_NCORES)),
                               trace=trace)
    out = np.concatenate([r["out"] for r in res.results], axis=0)
    return out.reshape(n, C, H, W), res


def _out_valid(out):
    """Sanity invariant: the final hardtanh clamps every output to [-1, 1].

    A fresh NEFF's first execution has been observed to return garbage
    (values way outside [-1, 1]) or raise a transient device error; the
    identical program then runs fine.  Values outside the clamp range (or
    non-finite) can only come from such a bad run.
    """
    return bool(np.isfinite(out).all() and np.abs(out).max() <= 1.0)


def kernel(x, w1, w2, g1, b1, m1, v1, g2, b2, m2, v2):
    import time

    out = None
    for attempt in range(3):
        try:
            out, _ = _run(x=x, w1=w1, w2=w2, g1=g1, b1=b1, m1=m1, v1=v1,
                          g2=g2, b2=b2, m2=m2, v2=v2)
        except Exception:
            if attempt == 2:
                raise
            time.sleep(2.0)
            continue
        if _out_valid(out):
            break
        time.sleep(2.0)
    return out


# revision 27
# speedup vs baseline: 1.0146x; 1.0053x over previous
"""Trainium2 Bass kernel for a binarized-weight BasicBlock (dense CNN).

Reference computation (all fp32):
    out = clip(bn2(conv3x3(quant(clip(bn1(conv3x3(quant(x), sign(w1))), -1, 1)),
                  sign(w2)) + x), -1, 1)
with quant(v) = round-half-up(v * 128) / 128 and bn in inference form.

Strategy:
  * Data-parallel: batch 32 is sharded 4 images per NeuronCore across 8 cores.
  * Channels (256) live on partitions as 2 blocks of 128.
  * conv1 = 18 accumulating fp16 matmuls per output tile (9 taps x 2 input
    channel blocks), fp16 operands / fp32 PSUM accumulation.  Activations are
    integers k = 128*quant(v) with |k| <= ~730 and weights are +-1, so every
    product and partial sum is exactly representable: the fp16 matmul path is
    bit-exact, and PSUM holds 128*conv exactly.
  * conv2 = 9 accumulating fp8e4 DoubleRow matmuls per output tile: the two
    128-channel input blocks are packed into the DoubleRow pair axis, so one
    matmul contracts all 256 channels of a tap at 1 output column per cycle
    (2x the fp16 rate).  conv2's input k2 = 128*quant(clip(bn1,...)) is an
    integer in [-128, 128]; 98.3% of values saturate to +-128 (exact in
    e4m3) and the rest round with |err| <= 4, giving ~1.1e-2 final relative
    error (vs the 2e-2 tolerance).  The moving AP is [128, 2, 8, 56]: pair
    axis stride PLANEP (%16, a DoubleRow requirement), then the usual
    strided row window.
  * Activations are staged in zero-padded SBUF tiles; a conv matmul's moving
    operand is a window into the padded plane, so no shift DMAs are needed.
  * quantize is exact: z = 128*v + 0.5 (exact in fp32), t = RNE(z) via the
    +-1.5*2^23 magic add, floor(z) = t - (t > z).  Matches the reference's
    round-half-up tie behaviour bit-for-bit.
  * BN is folded host-side to per-channel (inv, bias) fp32 pairs; the device
    applies psum*(inv/128) + bias with the same fp32 rounding sequence as the
    reference.
"""

import numpy as np

_N = 32          # full batch
_C = 256         # channels
_H = 56          # height
_W = 56          # width
_NCORES = 8
_EPS = 1e-5

_cache = {}


def _build(n_img, C, H, W, RG):
    """Build + compile the per-core Bass program (SPMD, one NEFF for all cores)."""
    from contextlib import ExitStack

    import concourse.tile as tile
    from concourse import bacc, mybir

    F32 = mybir.dt.float32
    F16 = mybir.dt.float16
    F8 = mybir.dt.float8e4
    Alu = mybir.AluOpType
    Act = mybir.ActivationFunctionType
    DR = mybir.MatmulPerfMode.DoubleRow

    MAGIC = float(3 << 22)  # 1.5 * 2**23: RNE-to-integer for |z| < 2**22

    nblk = C // 128
    ngrp = H // RG
    HP, WP = H + 2, W + 2
    NW = 9 * nblk * nblk          # weight tiles per conv (36)
    PLANE = HP * WP               # 3364
    PLANEP = (PLANE + 15) // 16 * 16   # 3376: padded plane, %16 pair stride
    P2SZ = nblk * PLANEP          # pad2 flat bytes per partition

    nc = bacc.Bacc("TRN2", target_bir_lowering=False, debug=False,
                   num_devices=_NCORES)

    x_d = nc.dram_tensor("x", [n_img, C, H * W], F32, kind="ExternalInput")
    w1_d = nc.dram_tensor("wq1", [128, NW, 128], F16, kind="ExternalInput")
    w2_d = nc.dram_tensor("wq2", [128, NW, 128], F8, kind="ExternalInput")
    c_d = nc.dram_tensor("coef", [128, nblk, 4], F32, kind="ExternalInput")
    o_d = nc.dram_tensor("out", [n_img, C, H * W], F32, kind="ExternalOutput")

    def quant_chain(pool, zsrc, zscale, dst_ap, shape):
        """dst = floor(zscale*zsrc + 0.5) as fp16; exact round-half-up."""
        z = pool.tile(shape, F32, tag="qz")
        nc.scalar.activation(z[:], zsrc, Act.Copy, bias=0.5, scale=zscale)
        t = pool.tile(shape, F32, tag="qt")
        nc.vector.tensor_scalar(t[:], z[:], MAGIC, -MAGIC, Alu.add, Alu.add)
        c = pool.tile(shape, F32, tag="qc")
        nc.vector.tensor_tensor(c[:], t[:], z[:], Alu.is_gt)
        nc.vector.tensor_tensor(dst_ap, t[:], c[:], Alu.subtract)

    with tile.TileContext(nc) as tc, ExitStack() as ctx:
        const = ctx.enter_context(tc.tile_pool(name="const", bufs=1))
        xin = ctx.enter_context(tc.tile_pool(name="xin", bufs=2))
        pads = ctx.enter_context(tc.tile_pool(name="pads", bufs=2))
        q1s = ctx.enter_context(tc.tile_pool(name="q1s", bufs=2))
        e1s = ctx.enter_context(tc.tile_pool(name="e1s", bufs=3))
        e2s = ctx.enter_context(tc.tile_pool(name="e2s", bufs=6))
        psum = ctx.enter_context(tc.tile_pool(name="psum", bufs=7, space="PSUM"))
        warmp = ctx.enter_context(tc.tile_pool(name="warmp", bufs=1,
                                               space="PSUM"))

        # weight tiles: conv1 fp16 [ob][tap][ib], conv2 fp8 DoubleRow pairs
        # [ob][tap][pair=ib].  The first 4 conv1 tiles go in a mini-DMA so the
        # warm-up matmuls can start early.
        wt1 = const.tile([128, NW, 128], F16)
        nc.sync.dma_start(wt1[:, 0:4, :], w1_d.ap()[:, 0:4, :])

        wt2 = const.tile([128, NW, 128], F8)

        ct = const.tile([128, nblk, 4], F32)

        # image 0 in three DMA chunks (rows 0-9 gate the first matmul
        # group); quant chains stay piece-granular for early starts
        xi0 = x_d.ap()[0].rearrange("(b p) f -> p b f", p=128)
        x0_pieces = [[(0, 4), (4, 4)],
                     [(0, 2), (2, 6)],
                     [(0, 2), (2, 6)]] + \
            [[(0, 4), (4, 4)]] * (ngrp - 3)
        x0_pieces = x0_pieces[:ngrp]
        xt0 = xin.tile([128, nblk, H * W], F32, tag="x", name="x0")

        def x0_dma(g, pr, pn):
            a = g * RG + pr
            nc.sync.dma_start(xt0[:, :, a * W:(a + pn) * W],
                              xi0[:, :, a * W:(a + pn) * W])

        # rows 0-9 gate the first matmul group; issue those pieces, then
        # the first conv's weight bulk, then the rest in row order
        early = [(0, pr, pn) for (pr, pn) in x0_pieces[0]]
        late = []
        if ngrp > 1:
            early.append((1, *x0_pieces[1][0]))
            late += [(1, pr, pn) for (pr, pn) in x0_pieces[1][1:]]
        for g in range(2, ngrp):
            late += [(g, pr, pn) for (pr, pn) in x0_pieces[g]]
        for (g, pr, pn) in early:
            x0_dma(g, pr, pn)
        nc.sync.dma_start(ct[:], c_d.ap())
        nc.sync.dma_start(wt1[:, 4:18, :], w1_d.ap()[:, 4:18, :])
        for (g, pr, pn) in late:
            x0_dma(g, pr, pn)
        nc.sync.dma_start(wt1[:, 18:NW, :], w1_d.ap()[:, 18:NW, :])
        nc.sync.dma_start(wt2[:, 0:18, :], w2_d.ap()[:, 0:18, :])
        nc.sync.dma_start(wt2[:, 18:NW, :], w2_d.ap()[:, 18:NW, :])

        # dummy matmuls on the first mini-chunk: keeps the PE activity
        # monitor busy during the input fill so the real stream starts at
        # the full 2.4GHz clock
        warm = warmp.tile([128, 128], F32)
        for j in range(65):
            nc.tensor.matmul(warm[:], wt1[:, 0, :], wt1[:, j % 4, :],
                             start=True, stop=True)

        def conv1_mms(ps, pad, ob, r0, rg):
            """18 accumulating fp16 matmuls for output block ob, rows r0:+rg."""
            for tap in range(9):
                dy, dx = tap // 3 - 1, tap % 3 - 1
                for ib in range(nblk):
                    widx = (ob * 9 + tap) * nblk + ib
                    rhs = pad[:, ib, 1 + r0 + dy:1 + r0 + dy + rg,
                              1 + dx:1 + dx + W]
                    nc.tensor.matmul(ps[:, :rg * W], wt1[:, widx, :], rhs,
                                     start=(tap == 0 and ib == 0),
                                     stop=(tap == 8 and ib == nblk - 1))

        for i in range(n_img):
            if i == 0:
                xt = xt0
            else:
                xi = x_d.ap()[i].rearrange("(b p) f -> p b f", p=128)
                xt = xin.tile([128, nblk, H * W], F32, tag="x")
                nc.sync.dma_start(xt[:], xi)

            def xg(g):
                return xt[:, :, g * RG * W:(g + 1) * RG * W]

            # quantize input into padded conv1 operand (fp16, 4D tile)
            pad1 = pads.tile([128, nblk, HP, WP], F16, tag="pad1")
            nc.vector.memset(pad1[:, :, 0, :], 0.0)
            nc.vector.memset(pad1[:, :, HP - 1, :], 0.0)
            nc.vector.memset(pad1[:, :, 1:HP - 1, 0:1], 0.0)
            nc.vector.memset(pad1[:, :, 1:HP - 1, WP - 1:WP], 0.0)
            for g in range(ngrp):
                if i == 0:
                    # pieces matching the split DMAs
                    for (pr, pn) in x0_pieces[g]:
                        rows = slice(pr * W, (pr + pn) * W)
                        dst = pad1[:, :,
                                   1 + g * RG + pr:1 + g * RG + pr + pn,
                                   1:1 + W]
                        quant_chain(
                            q1s,
                            xg(g)[:, :, rows].rearrange(
                                "p b (h w) -> p b h w", w=W),
                            128.0, dst, [128, nblk, pn, W])
                else:
                    dst = pad1[:, :, 1 + g * RG:1 + (g + 1) * RG, 1:1 + W]
                    quant_chain(
                        q1s,
                        xg(g).rearrange("p b (h w) -> p b h w", w=W),
                        128.0, dst, [128, nblk, RG, W])

            # conv2 operand: fp8 planes at %16 pair stride (DoubleRow req.)
            pad2 = pads.tile([128, P2SZ], F8, tag="pad2")
            p2v4 = pad2[:].rearrange("p (b f) -> p b f", b=nblk)[
                :, :, 0:PLANE].rearrange("p b (h w) -> p b h w", w=WP)
            nc.vector.memset(p2v4[:, :, 0, :], 0.0)
            nc.vector.memset(p2v4[:, :, HP - 1, :], 0.0)
            nc.vector.memset(p2v4[:, :, 1:HP - 1, 0:1], 0.0)
            nc.vector.memset(p2v4[:, :, 1:HP - 1, WP - 1:WP], 0.0)

            # conv1 -> bn1 -> hardtanh -> quantize into fp8 conv2 operand
            grps1 = [(g * RG, RG) for g in range(ngrp)]
            for ob in range(nblk):
                for (r0, rg) in grps1:
                    ps = psum.tile([128, RG * W], F32, tag="ps")
                    conv1_mms(ps, pad1, ob, r0, rg)
                    psf = ps[:, :rg * W]
                    # z = 128*bn1 + 0.5 in one ACT op (psum = 128*conv, so
                    # scale=inv1, bias=128*bias1+0.5); matches XLA's fused
                    # multiply-add rounding bit-for-bit
                    n = rg * W
                    z = e1s.tile([128, RG * W], F32, tag="zb")
                    nc.scalar.activation(z[:, :n], psf, Act.Identity,
                                         bias=ct[:, ob, 1:2],
                                         scale=ct[:, ob, 0:1])
                    cl = e1s.tile([128, RG * W], F32, tag="cl")
                    nc.vector.tensor_scalar(cl[:, :n], z[:, :n], 128.5,
                                            -127.5, Alu.min, Alu.max)
                    t = e1s.tile([128, RG * W], F32, tag="qt")
                    nc.vector.tensor_scalar(t[:, :n], cl[:, :n], MAGIC,
                                            -MAGIC, Alu.add, Alu.add)
                    c = e1s.tile([128, RG * W], F32, tag="qc")
                    nc.vector.tensor_tensor(c[:, :n], t[:, :n], cl[:, :n],
                                            Alu.is_gt)
                    # fp8 cast on write (RNE); |k2| <= 128 < 240 never clips
                    dst = p2v4[:, ob, 1 + r0:1 + r0 + rg, 1:1 + W]
                    nc.vector.tensor_tensor(
                        dst, t[:, :n].rearrange("p (h w) -> p h w", w=W),
                        c[:, :n].rearrange("p (h w) -> p h w", w=W),
                        Alu.subtract)

            # conv2 (fp8 DoubleRow) -> +residual -> bn2 -> hardtanh -> out
            # moving AP: [128, 2(pair), rg, W] strided row windows
            for ob in range(nblk):
                for g in range(ngrp):
                    r0 = g * RG
                    ps = psum.tile([128, RG * W], F32, tag="ps")
                    for tap in range(9):
                        dy, dx = tap // 3 - 1, tap % 3 - 1
                        rhs = p2v4[:, :, 1 + r0 + dy:1 + r0 + dy + RG,
                                   1 + dx:1 + dx + W]
                        nc.tensor.matmul(
                            ps[:], wt2[:, (ob * 9 + tap) * nblk:
                                       (ob * 9 + tap) * nblk + nblk, :],
                            rhs, start=(tap == 0), stop=(tap == 8),
                            perf_mode=DR)
                    psf = ps[:]
                    res = xg(g)[:, ob, :]
                    bn = e2s.tile([128, RG * W], F32, tag="bn2")
                    oc = e2s.tile([128, RG * W], F32, tag="oc")
                    last = (i == n_img - 1 and ob == nblk - 1
                            and g == ngrp - 1)
                    if not last:
                        s_t = e2s.tile([128, RG * W], F32, tag="s")
                        nc.vector.scalar_tensor_tensor(
                            s_t[:], psf, 1.0 / 128.0, res,
                            Alu.mult, Alu.add)
                        nc.scalar.activation(bn[:], s_t[:], Act.Identity,
                                             bias=ct[:, ob, 3:4],
                                             scale=ct[:, ob, 2:3])
                        nc.vector.tensor_scalar(oc[:], bn[:],
                                                1.0, -1.0, Alu.min, Alu.max)
                        nc.sync.dma_start(
                            o_d.ap()[i, ob * 128:(ob + 1) * 128,
                                     g * RG * W:(g + 1) * RG * W],
                            oc[:])
                    else:
                        # exposed kernel tail: keep ACT off the critical
                        # path by pre-folding bn2 into the residual
                        # (resb = x*inv2 + bias2, computed during the MMs),
                        # then psum*(inv2/128) + resb -> clip -> DMA in
                        # halves.  Ulp-level deviation from the reference
                        # rounding sequence, far below tolerance.
                        resb = e2s.tile([128, RG * W], F32, tag="resb")
                        nc.scalar.activation(resb[:], res, Act.Identity,
                                             bias=ct[:, ob, 3:4],
                                             scale=ct[:, ob, 2:3])
                        iv = e2s.tile([128, 1], F32, tag="iv")
                        nc.vector.tensor_scalar(iv[:], ct[:, ob, 2:3],
                                                1.0 / 128.0, None, Alu.mult)
                        nh = RG * W // 2
                        for hsl in (slice(0, nh), slice(nh, RG * W)):
                            nc.vector.scalar_tensor_tensor(
                                bn[:, hsl], psf[:, hsl], iv[:],
                                resb[:, hsl], Alu.mult, Alu.add)
                            nc.vector.tensor_scalar(
                                oc[:, hsl], bn[:, hsl],
                                1.0, -1.0, Alu.min, Alu.max)
                            nc.sync.dma_start(
                                o_d.ap()[i, ob * 128:(ob + 1) * 128,
                                         g * RG * W:(g + 1) * RG * W][:, hsl],
                                oc[:, hsl])

    nc.compile()
    return nc


def _get_program(n_img, C, H, W, RG):
    key = (n_img, C, H, W, RG)
    if key not in _cache:
        _cache[key] = _build(n_img, C, H, W, RG)
    return _cache[key]


def _fold_bn(g, b, m, v):
    """Per-channel (inv, bias) in fp32, matching the reference's op sequence."""
    try:
        import jax

        with jax.default_device(jax.devices("cpu")[0]):
            inv = np.asarray(jax.jit(
                lambda g_, v_: g_ * jax.lax.rsqrt(v_ + _EPS), backend="cpu"
            )(g, v))
            bias = np.asarray(jax.jit(
                lambda b_, m_, i_: b_ - m_ * i_, backend="cpu"
            )(b, m, inv))
        return inv.astype(np.float32), bias.astype(np.float32)
    except Exception:
        inv = (g.astype(np.float32)
               * (np.float32(1.0) / np.sqrt(v.astype(np.float32)
                                            + np.float32(_EPS))))
        bias = b.astype(np.float32) - m.astype(np.float32) * inv
        return inv.astype(np.float32), bias.astype(np.float32)


def _prep_weights(w1, w2, C):
    """lhsT tiles [128, 36, 128], slot (ob*9+tap)*nblk+ib, i on partitions."""
    import ml_dtypes

    nblk = C // 128
    t1 = np.empty((128, 9 * nblk * nblk, 128), np.float16)
    t2 = np.empty((128, 9 * nblk * nblk, 128), ml_dtypes.float8_e4m3)
    for w, tiles, dt in ((w1, t1, np.float16),
                        (w2, t2, ml_dtypes.float8_e4m3)):
        wq = np.where(w >= 0, 1.0, -1.0).astype(np.float32)
        for ob in range(nblk):
            for tap in range(9):
                dy, dx = tap // 3, tap % 3
                for ib in range(nblk):
                    idx = (ob * 9 + tap) * nblk + ib
                    blk = wq[ob * 128:(ob + 1) * 128,
                             ib * 128:(ib + 1) * 128, dy, dx]
                    tiles[:, idx, :] = blk.T.astype(dt)
    return t1, t2


def _make_in_maps(x, w1, w2, g1, b1, m1, v1, g2, b2, m2, v2):
    n, C, H, W = x.shape
    n_img = n // _NCORES
    nblk = C // 128

    wq1, wq2 = _prep_weights(np.asarray(w1), np.asarray(w2), C)
    inv1, bias1 = _fold_bn(np.asarray(g1), np.asarray(b1),
                           np.asarray(m1), np.asarray(v1))
    inv2, bias2 = _fold_bn(np.asarray(g2), np.asarray(b2),
                           np.asarray(m2), np.asarray(v2))
    bias1z = np.float32(128.0) * bias1 + np.float32(0.5)
    coef = np.empty((128, nblk, 4), np.float32)
    for blk in range(nblk):
        sl = slice(blk * 128, (blk + 1) * 128)
        coef[:, blk, 0] = inv1[sl]
        coef[:, blk, 1] = bias1z[sl]
        coef[:, blk, 2] = inv2[sl]
        coef[:, blk, 3] = bias2[sl]

    xr = np.ascontiguousarray(np.asarray(x).reshape(n, C, H * W),
                              dtype=np.float32)
    return [
        {"x": xr[i * n_img:(i + 1) * n_img], "wq1": wq1, "wq2": wq2,
         "coef": coef}
        for i in range(_NCORES)
    ]


def _run(trace=False, **inputs):
    from concourse.bass_utils import run_bass_kernel_spmd

    n, C, H, W = inputs["x"].shape
    nc = _get_program(n // _NCORES, C, H, W, 8)
    in_maps = _make_in_maps(**inputs)
    res = run_bass_kernel_spmd(nc, in_maps, core_ids=list(range(_NCORES)),
                               trace=trace)
    out = np.concatenate([r["out"] for r in res.results], axis=0)
    return out.reshape(n, C, H, W), res


def _out_valid(out):
    """Sanity invariant: the final hardtanh clamps every output to [-1, 1].

    A fresh NEFF's first execution has been observed to return garbage
    (values way outside [-1, 1]) or raise a transient device error; the
    identical program then runs fine.  Values outside the clamp range (or
    non-finite) can only come from such a bad run.
    """
    return bool(np.isfinite(out).all() and np.abs(out).max() <= 1.0)


def kernel(x, w1, w2, g1, b1, m1, v1, g2, b2, m2, v2):
    import time

    out = None
    for attempt in range(3):
        try:
            out, _ = _run(x=x, w1=w1, w2=w2, g1=g1, b1=b1, m1=m1, v1=v1,
                          g2=g2, b2=b2, m2=m2, v2=v2)
        except Exception:
            if attempt == 2:
                raise
            time.sleep(2.0)
            continue
        if _out_valid(out):
            break
        time.sleep(2.0)
    return out
